# revision 24
# baseline (speedup 1.0000x reference)
"""Trainium2 Bass kernel for nn_LMDecoder (embedding -> degenerate GRU cell -> vocab classifier).

Computation (per reference):
    x  = embedding[target_sequence]              # [B, T, E]
    gi = x @ w_ih.T + b_ih                       # [B, T, 3H]
    r  = sigmoid(i_r + b_hr); z = sigmoid(i_z + b_hz)
    n  = tanh(i_n + r * b_hn)
    h  = (1 - z) * n                             # [B, T, H]
    logits = h @ w_cls.T + b_cls                 # [B, T, V]

Sharding: data-parallel over batch across 8 cores (B=64 -> 8 rows/core ->
M=1024 tokens/core). Device kernel computes logits transposed (vocab on
SBUF partitions) so b_cls rides as a per-partition bias on PSUM eviction;
output DRAM layout is [128, NV, M] (partition-major) so multi-tile stores
are contiguous per partition; host reassembles.

Matmuls run in fp16 with f32 PSUM accumulation and fp16 output (measured
3.7e-4 relative error vs the f32 reference on hardware).
"""

import sys

sys.path.insert(0, "/opt/trn_rl_repo")

from contextlib import ExitStack

import numpy as np

import concourse.bacc as bacc
import concourse.mybir as mybir
import concourse.tile as tile
from concourse.bass_utils import run_bass_kernel_spmd

FP16 = mybir.dt.float16
F32 = mybir.dt.float32
AF = mybir.ActivationFunctionType

V, E, H, B, T = 32000, 256, 256, 64, 128
N_CORES = 8
M = (B // N_CORES) * T  # tokens per core = 1024
NV = V // 128  # 250 vocab tiles
VG = 8  # vocab tiles per w_cls DMA load group
SG = 4  # vocab tiles per output store
HALF = M // 2  # 512
N_EARLY = 4  # w_cls load groups pulled onto the sync queue up front


def _build_program():
    nc = bacc.Bacc(
        "TRN2",
        target_bir_lowering=False,
        debug=False,
        num_devices=N_CORES,
    )

    xT = nc.dram_tensor("xT", [E, M], FP16, kind="ExternalInput").ap()
    w_ihT = nc.dram_tensor("w_ihT", [E, 3 * H], FP16, kind="ExternalInput").ap()
    w_clsT = nc.dram_tensor("w_clsT", [H, V], FP16, kind="ExternalInput").ap()
    # per-partition bias columns: 0..5 = gate biases for gi^T partition tiles
    # (r0,r1,z0,z1,n0,n1), 6..7 = b_hn for h tiles 0,1
    b_misc = nc.dram_tensor("b_misc", [128, 8], F32, kind="ExternalInput").ap()
    # b_cls tiled: column v = bias for vocab partition tile v
    b_cls_t = nc.dram_tensor("b_cls_t", [128, NV], F32, kind="ExternalInput").ap()
    # logits, vocab-tiled partition-major: [p, v, m] = logit(token m, vocab v*128+p)
    logits3 = nc.dram_tensor("logits3", [128, NV, M], FP16, kind="ExternalOutput").ap()

    with tile.TileContext(nc) as tc, ExitStack() as ctx:
        const_pool = ctx.enter_context(tc.tile_pool(name="const", bufs=1))
        gru_pool = ctx.enter_context(tc.tile_pool(name="gru", bufs=1))
        w_pool = ctx.enter_context(tc.tile_pool(name="wcls", bufs=6))
        we_pool = ctx.enter_context(tc.tile_pool(name="wcls_early", bufs=1))
        out_pool = ctx.enter_context(tc.tile_pool(name="out", bufs=4))
        psum_pool = ctx.enter_context(tc.tile_pool(name="psum", bufs=4, space="PSUM"))

        # ---- load constants / activations (before the wk prefetch so the
        # GRU matmuls aren't queued behind 1MB of w_cls on the sync ring) ----
        x0 = const_pool.tile([128, M], FP16, tag="x0")
        x1 = const_pool.tile([128, M], FP16, tag="x1")
        nc.sync.dma_start(out=x0[:], in_=xT[0:128, :])
        nc.sync.dma_start(out=x1[:], in_=xT[128:256, :])
        wih0 = const_pool.tile([128, 3 * H], FP16, tag="wih0")
        wih1 = const_pool.tile([128, 3 * H], FP16, tag="wih1")
        nc.sync.dma_start(out=wih0[:], in_=w_ihT[0:128, :])
        nc.sync.dma_start(out=wih1[:], in_=w_ihT[128:256, :])
        bm = const_pool.tile([128, 8], F32, tag="bm")
        nc.sync.dma_start(out=bm[:], in_=b_misc[:, :])
        bc = const_pool.tile([128, NV], F32, tag="bc")
        nc.sync.dma_start(out=bc[:], in_=b_cls_t[:, :])

        # ---- early w_cls prefetch on the (otherwise idle) sync HWDGE queue ----
        early_wk = []
        for eg in range(N_EARLY):
            vs = slice(eg * VG * 128, (eg + 1) * VG * 128)
            ewk0 = we_pool.tile([128, VG * 128], FP16, tag=f"ewk0_{eg}", name=f"ewk0_{eg}")
            ewk1 = we_pool.tile([128, VG * 128], FP16, tag=f"ewk1_{eg}", name=f"ewk1_{eg}")
            nc.sync.dma_start(out=ewk0[:], in_=w_clsT[0:128, vs])
            nc.sync.dma_start(out=ewk1[:], in_=w_clsT[128:256, vs])
            early_wk.append((ewk0, ewk1))

        # ---- PE warmup: a burst of junk LDWEIGHTS keeps the PE array busy
        # from the moment x0 lands so the HAM clock gate reaches 8/8 just as
        # the (otherwise cold) GRU matmuls issue.  LDWEIGHTS writes no PSUM,
        # takes no pool slot, and the loaded weights are overwritten by the
        # first real matmul.
        for _ in range(24):
            nc.tensor.ldweights(x0[:, 0:128])

        # ---- GRU gates: gi^T tiles [128, M], partition tiles g=0..5 ----
        # g=0,1 -> i_r (h dims 0..255), g=2,3 -> i_z, g=4,5 -> i_n.
        # Uses 1-sigmoid(x) = sigmoid(-x): zc = sigmoid(-(i_z + b_z)) so
        # h = zc * n with no extra subtract; b_misc cols 2,3 hold -(b_z).
        r_sb = [gru_pool.tile([128, M], F32, tag=f"r{i}", name=f"r{i}") for i in range(2)]
        zc_sb = [gru_pool.tile([128, M], F32, tag=f"zc{i}", name=f"zc{i}") for i in range(2)]
        n_sb = [gru_pool.tile([128, M], F32, tag=f"n{i}", name=f"n{i}") for i in range(2)]
        s_sb = [gru_pool.tile([128, M], F32, tag=f"s{i}", name=f"s{i}") for i in range(2)]
        h_sb = [gru_pool.tile([128, M], FP16, tag=f"h{i}", name=f"h{i}") for i in range(2)]

        def gate_psum(g):
            """Matmul gi^T partition tile g into one [128, M] psum tile (2 banks)."""
            ps = psum_pool.tile([128, M], F32, tag="ps", name="ps")
            for c in range(2):
                cs = slice(c * HALF, (c + 1) * HALF)
                nc.tensor.matmul(
                    ps[:, cs],
                    lhsT=wih0[:, g * 128 : (g + 1) * 128],
                    rhs=x0[:, cs],
                    start=True,
                    stop=False,
                )
                nc.tensor.matmul(
                    ps[:, cs],
                    lhsT=wih1[:, g * 128 : (g + 1) * 128],
                    rhs=x1[:, cs],
                    start=False,
                    stop=True,
                )
            return ps

        for i in range(2):
            ps = gate_psum(i)
            nc.scalar.activation(r_sb[i][:], ps[:], AF.Sigmoid, bias=bm[:, i : i + 1])
            ps = gate_psum(2 + i)
            nc.scalar.activation(
                zc_sb[i][:], ps[:], AF.Sigmoid, bias=bm[:, 2 + i : 3 + i], scale=-1.0
            )
            ps = gate_psum(4 + i)
            # s = i_n + r * b_hn  (fused); n = tanh(s + b_in); h = zc * n
            nc.vector.scalar_tensor_tensor(
                s_sb[i][:], r_sb[i][:], bm[:, 6 + i : 7 + i], ps[:],
                op0=mybir.AluOpType.mult, op1=mybir.AluOpType.add,
            )
            nc.scalar.activation(
                n_sb[i][:], s_sb[i][:], AF.Tanh, bias=bm[:, 4 + i : 5 + i]
            )
            nc.vector.tensor_mul(h_sb[i][:], zc_sb[i][:], n_sb[i][:])

        # ---- classifier: logits tile v = (w_cls tile v) @ h + b_cls ----
        # PE: per vocab tile, 4 matmuls (2 k-slices x 2 token halves) accumulate
        # into one [128, M] psum tile (2 banks).  Eviction: one whole-tile
        # f32->fp16 op with per-partition b_cls bias, alternating ACT / DVE.
        # Stores: SG tiles batched per DMA into the [128, NV, M] layout.
        ot = None
        for vg in range(0, NV, VG):
            ntile = min(VG, NV - vg)
            eg = vg // VG
            if eg < N_EARLY:
                wk0, wk1 = early_wk[eg]
            else:
                ncols = ntile * 128
                wk0 = w_pool.tile([128, VG * 128], FP16, tag="wk0")
                wk1 = w_pool.tile([128, VG * 128], FP16, tag="wk1")
                vs = slice(vg * 128, vg * 128 + ncols)
                nc.gpsimd.dma_start(out=wk0[:, 0:ncols], in_=w_clsT[0:128, vs])
                nc.gpsimd.dma_start(out=wk1[:, 0:ncols], in_=w_clsT[128:256, vs])
            for vi in range(ntile):
                v = vg + vi
                si = v % SG
                if si == 0:
                    ot = out_pool.tile([128, SG * M], FP16, tag="ot")
                ws = slice(vi * 128, (vi + 1) * 128)
                ps = psum_pool.tile([128, M], F32, tag="ps", name="ps")
                nc.tensor.matmul(
                    ps[:, 0:HALF], lhsT=wk0[:, ws], rhs=h_sb[0][:, 0:HALF],
                    start=True, stop=False,
                )
                nc.tensor.matmul(
                    ps[:, HALF:M], lhsT=wk0[:, ws], rhs=h_sb[0][:, HALF:M],
                    start=True, stop=False,
                )
                nc.tensor.matmul(
                    ps[:, 0:HALF], lhsT=wk1[:, ws], rhs=h_sb[1][:, 0:HALF],
                    start=False, stop=True,
                )
                nc.tensor.matmul(
                    ps[:, HALF:M], lhsT=wk1[:, ws], rhs=h_sb[1][:, HALF:M],
                    start=False, stop=True,
                )
                dst = ot[:, si * M : (si + 1) * M]
                if v % 2 == 0:
                    nc.scalar.activation(
                        dst, ps[:], AF.Identity, bias=bc[:, v : v + 1]
                    )
                else:
                    nc.vector.tensor_scalar_add(dst, ps[:], bc[:, v : v + 1])
                if v >= ((NV - 1) // SG) * SG - SG:
                    # drain the last two groups tile-by-tile to shorten the tail
                    nc.sync.dma_start(
                        out=logits3[:, v : v + 1, :],
                        in_=ot[:, si * M : (si + 1) * M],
                    )
                elif si == SG - 1:
                    v0 = v - si
                    nc.sync.dma_start(
                        out=logits3[:, v0 : v + 1, :], in_=ot[:, 0 : (si + 1) * M]
                    )

    nc.compile()
    return nc


_NC_CACHE = None


def _get_program():
    global _NC_CACHE
    if _NC_CACHE is None:
        _NC_CACHE = _build_program()
    return _NC_CACHE


def _prep_in_maps(
    target_sequence, embedding, w_ih, b_ih, b_hh, w_cls, b_cls
) -> list[dict]:
    embedding = np.asarray(embedding, np.float32)
    w_ih = np.asarray(w_ih, np.float32)
    b_ih = np.asarray(b_ih, np.float32)
    b_hh = np.asarray(b_hh, np.float32)
    w_cls = np.asarray(w_cls, np.float32)
    b_cls = np.asarray(b_cls, np.float32)
    seq = np.asarray(target_sequence).astype(np.int64)

    # shared (identical on every core) tensors
    w_ihT = np.ascontiguousarray(w_ih.T).astype(np.float16)  # [E, 3H]
    w_clsT = np.ascontiguousarray(w_cls.T).astype(np.float16)  # [H, V]
    b_misc = np.zeros((128, 8), np.float32)
    b_rz = (b_ih[: 2 * H] + b_hh[: 2 * H]).reshape(4, 128)  # r0 r1 z0 z1
    b_misc[:, 0:2] = b_rz[0:2].T
    b_misc[:, 2:4] = -b_rz[2:4].T  # negated: zc = sigmoid(-(i_z + b_z))
    b_misc[:, 4:6] = b_ih[2 * H :].reshape(2, 128).T  # b_in
    b_misc[:, 6:8] = b_hh[2 * H :].reshape(2, 128).T  # b_hn
    b_cls_t = np.ascontiguousarray(b_cls.reshape(NV, 128).T)  # [128, NV]

    rows_per_core = B // N_CORES
    in_maps = []
    for c in range(N_CORES):
        toks = seq[c * rows_per_core : (c + 1) * rows_per_core].reshape(-1)  # [M]
        x = embedding[toks]  # [M, E] f32
        xT = np.ascontiguousarray(x.T).astype(np.float16)  # [E, M]
        in_maps.append(
            {
                "xT": xT,
                "w_ihT": w_ihT,
                "w_clsT": w_clsT,
                "b_misc": b_misc,
                "b_cls_t": b_cls_t,
            }
        )
    return in_maps


def _assemble(results) -> np.ndarray:
    rows_per_core = B // N_CORES
    out = np.empty((B, T, V), np.float32)
    for c in range(N_CORES):
        lt = results[c]["logits3"]  # [128, NV, M] fp16
        # logits[token m, vocab v*128+p] = lt[p, v, m]
        out[c * rows_per_core : (c + 1) * rows_per_core] = (
            lt.transpose(2, 1, 0).reshape(M, V).reshape(rows_per_core, T, V)
        )
    return out


def kernel(
    target_sequence: np.ndarray,
    embedding: np.ndarray,
    w_ih: np.ndarray,
    b_ih: np.ndarray,
    b_hh: np.ndarray,
    w_cls: np.ndarray,
    b_cls: np.ndarray,
) -> np.ndarray:
    in_maps = _prep_in_maps(
        target_sequence, embedding, w_ih, b_ih, b_hh, w_cls, b_cls
    )
    nc = _get_program()
    res = run_bass_kernel_spmd(nc, in_maps, list(range(N_CORES)))
    return _assemble(res.results)


def run_profiled(inputs: dict, tmpdir: str | None = None):
    """Run with NTFF tracing; returns BassKernelResults (exec_time_ns etc.)."""
    in_maps = _prep_in_maps(**inputs)
    nc = _get_program()
    res = run_bass_kernel_spmd(
        nc, in_maps, list(range(N_CORES)), trace=True, tmpdir=tmpdir
    )
    return res
